# revision 1
# baseline (speedup 1.0000x reference)
"""DetectionLoss Trainium2 kernel (8-core data parallel).

Reference computation (per sample):
  decode 1176 pred boxes -> IoU vs 64 targets -> argmax over preds ->
  smooth-l1 on matched boxes + CE on matched class logits + BCE(conf, pos mask).
Output: scalar loss = (5*box + cls + conf) / 512.

Layout: groups of 2 samples; partitions = 2 x 64 targets; free dim = preds.
PE builds rank-2 "broadcast" matrices (i1 = bx2-tx1, i2 = tx2-bx1, S = ab+at,
wb, hb) in fp32r; ACT relu-evacuates; intersection dx = min(relu(i1),
relu(i2), wb, wt); score = dx*dy/(ab+at) which is monotone in IoU, so the
hardware max/max_index (first-occurrence) reproduces jnp.argmax exactly,
including all-zero-IoU ties.
"""

import numpy as np
from contextlib import ExitStack

import concourse.bass as bass
import concourse.mybir as mybir
from concourse import bacc, tile
from concourse.bass_utils import run_bass_kernel_spmd
from concourse.masks import make_identity

F32 = mybir.dt.float32
F32R = mybir.dt.float32r
I32 = mybir.dt.int32
U32 = mybir.dt.uint32
OP = mybir.AluOpType
AF = mybir.ActivationFunctionType
AX = mybir.AxisListType

B, N, T, C = 512, 1176, 64, 4
NCORES = 8
BC = B // NCORES          # samples per core = 64
NG = BC // 2              # groups of 2 samples = 32
NJ = 10                   # n tiles of 128 (padded)
NN = NJ * 128             # 1280
NTAIL = N - 9 * 128       # 24 valid rows in tile j=9
CHUNKS = [(i * 256, 256) for i in range(5)]
IMG_W, IMG_H = 1472.0, 832.0
LN16 = float(np.log(np.float64(16.0)))
SQRT_HALF = float(np.sqrt(np.float64(0.5)))

# pred-row quantity order (PE rhs rows); const row value per quantity
#   q0 I1x = bx2 - tx1   rhs=BX2,  const=-1, lhsT2=tx1
#   q1 I2x = tx2 - bx1   rhs=NBX1, const=+1, lhsT2=tx2
#   q2 I1y = by2 - ty1   rhs=BY2,  const=-1, lhsT2=ty1
#   q3 I2y = ty2 - by1   rhs=NBY1, const=+1, lhsT2=ty2
#   q4 SAB = ab + at     rhs=AB,   const=+1, lhsT2=at
#   q5 WBt = wb          rhs=WB,   const=0
#   q6 HBt = hb          rhs=HB,   const=0
NQ = 7
QCONST = [-1.0, 1.0, -1.0, 1.0, 1.0, 0.0, 0.0]

USE_GPSIMD = False
DEBUG = False
import os
STAGE = int(os.environ.get("KSTAGE", "4"))
KSUB = int(os.environ.get("KSUB", "9"))


def r32(ap):
    return ap.bitcast(F32R)


def build_kernel():
    nc = bacc.Bacc(
        "TRN2",
        target_bir_lowering=False,
        debug=False,
        enable_asserts=False,
        num_devices=NCORES,
    )
    pred_d = nc.dram_tensor("predictions", [BC, N, 9], F32, kind="ExternalInput").ap()
    tb_d = nc.dram_tensor("target_boxes", [BC, T, 4], F32, kind="ExternalInput").ap()
    tc_d = nc.dram_tensor("target_classes", [BC, T], I32, kind="ExternalInput").ap()
    ir_d = nc.dram_tensor(
        "initrows", [3, NQ * NN + NQ * 128], F32R, kind="ExternalInput"
    ).ap()
    out_d = nc.dram_tensor("out", [3], F32, kind="ExternalOutput").ap()
    dbg_d = nc.dram_tensor("dbg", [128, NG], F32, kind="ExternalOutput").ap() if DEBUG else None
    dbgs_d = nc.dram_tensor("dbgS", [NG, 128, NN], F32, kind="ExternalOutput").ap() if DEBUG else None

    with tile.TileContext(nc) as tcx:
        with ExitStack() as ctx:
            emit(ctx, tcx, pred_d, tb_d, tc_d, ir_d, out_d, dbg_d, dbgs_d)
    nc.compile()
    return nc


def emit(ctx, tcx, pred_d, tb_d, tc_d, ir_d, out_d, dbg_d, dbgs_d):
    nc = tcx.nc
    tp = lambda name, bufs, **kw: ctx.enter_context(
        tcx.tile_pool(name=name, bufs=bufs, **kw)
    )

    const_p = tp("const", 1)
    big_p = tp("big", 1)
    rows_p = tp("rows", 1)
    work_p = tp("work", 3)
    sbig_p = tp("sbig", 2)
    small_p = tp("small", 4)
    psA_p = tp("psA", 2, space="PSUM")      # (q0,q1),(q2,q3) double-buffered
    psC_p = tp("psC", 1, space="PSUM")      # q4 parity halves + (q5,q6) bank
    psB_p = tp("psB", 1, space="PSUM")      # prep transposes | MB+gather bank

    vec = nc.vector
    act = nc.scalar
    gps = nc.gpsimd if USE_GPSIMD else nc.vector

    # ---------------- constants ----------------
    ident = const_p.tile([128, 128], F32, tag="ident")
    make_identity(nc, ident[:, :])
    ones1 = const_p.tile([1, 128], F32, tag="ones1")
    nc.vector.memset(ones1[:, :], 1.0)
    onescol = const_p.tile([128, 1], F32, tag="onescol")
    nc.vector.memset(onescol[:, :], 1.0)
    njcol_i = const_p.tile([128, NJ], I32, tag="njcol_i")
    nc.gpsimd.iota(njcol_i[:, :], pattern=[[128, NJ]], base=0, channel_multiplier=1)
    njcol = const_p.tile([128, NJ], F32, tag="njcol")
    vec.tensor_copy(njcol[:, :], njcol_i[:, :])
    cbias = const_p.tile([128, 4], F32, tag="cbias")
    nc.vector.memset(cbias[:, 0:1], LN16)
    nc.vector.memset(cbias[:, 1:2], -IMG_W / 2)
    nc.vector.memset(cbias[:, 2:3], -IMG_H / 2)
    nc.vector.memset(cbias[:, 3:4], -1.0)
    b_ln16, b_wneg, b_hneg, b_neg1 = (cbias[:, i : i + 1] for i in range(4))  # noqa

    # ---------------- stage 0: loads ----------------
    # X[p, s, j, k] = pred[s, j*128+p, k]; pad rows zeroed, conf col -> -100
    X = big_p.tile([128, BC, NJ, 9], F32, tag="X")
    nc.vector.memset(X[:, :, 9, :], 0.0)
    nc.vector.memset(X[:, :, 9, 4], -100.0)
    nsplit = 2
    sw = BC // nsplit
    for i in range(nsplit):
        s0 = i * sw
        for j in range(NJ):
            pw = 128 if j < 9 else NTAIL
            eng = [nc.sync, nc.gpsimd, nc.scalar][(i * NJ + j) % 3]
            eng.dma_start(
                X[0:pw, s0 : s0 + sw, j, :],
                pred_d[s0 : s0 + sw, j * 128 : j * 128 + pw, :].rearrange(
                    "s p k -> p s k"
                ),
            )

    # targets: TGTC[p=(s,t), g, c], TCI[p, g]
    TGTC = big_p.tile([128, NG, 4], F32, tag="TGTC")
    nc.sync.dma_start(TGTC[:, :, :], tb_d.rearrange("(g s) t c -> (s t) g c", s=2))
    TCI = big_p.tile([128, NG], I32, tag="TCI")
    nc.sync.dma_start(TCI[:, :], tc_d.rearrange("(g s) t -> (s t) g", s=2))
    TCF = big_p.tile([128, NG], F32, tag="TCF")
    vec.tensor_copy(TCF[:, :], TCI[:, :])

    # ---------------- stage 0: decode ----------------
    # DQ[p, s, q, j]: q in (BX2, NBX1, BY2, NBY1, AB, WB, HB)
    DQ = big_p.tile([128, BC, NQ, NJ], F32, tag="DQ")
    WHX = big_p.tile([128, BC, NJ], F32, tag="WHX")
    WHY = big_p.tile([128, BC, NJ], F32, tag="WHY")
    CXY = big_p.tile([128, 2, BC, NJ], F32, tag="CXY")
    act.activation(WHX[:, :, :], X[:, :, :, 2], AF.Exp, bias=b_ln16, scale=1.0)
    act.activation(WHY[:, :, :], X[:, :, :, 3], AF.Exp, bias=b_ln16, scale=1.0)
    act.activation(
        CXY[:, 0, :, :], X[:, :, :, 0], AF.Identity, bias=b_wneg, scale=IMG_W
    )
    act.activation(
        CXY[:, 1, :, :], X[:, :, :, 1], AF.Identity, bias=b_hneg, scale=IMG_H
    )
    vec.tensor_tensor(DQ[:, :, 0, :], CXY[:, 0, :, :], WHX[:, :, :], OP.add)
    vec.tensor_tensor(DQ[:, :, 1, :], WHX[:, :, :], CXY[:, 0, :, :], OP.subtract)
    vec.tensor_tensor(DQ[:, :, 2, :], CXY[:, 1, :, :], WHY[:, :, :], OP.add)
    vec.tensor_tensor(DQ[:, :, 3, :], WHY[:, :, :], CXY[:, 1, :, :], OP.subtract)
    # q4 = AB (for SAB), q5 = WB, q6 = HB -- must match QCONST/lq mapping
    vec.tensor_tensor(DQ[:, :, 5, :], DQ[:, :, 0, :], DQ[:, :, 1, :], OP.add)
    vec.tensor_tensor(DQ[:, :, 6, :], DQ[:, :, 2, :], DQ[:, :, 3, :], OP.add)
    vec.tensor_tensor(DQ[:, :, 4, :], DQ[:, :, 5, :], DQ[:, :, 6, :], OP.mult)

    if STAGE == 1:
        o1 = small_p.tile([3, 1], F32, tag="o1")
        vec.tensor_copy(o1[:, :], DQ[0:3, 0, 0, 0:1])
        nc.sync.dma_start(out_d[:].rearrange("(x o) -> x o", o=1), o1[:, :])
        return

    # target-derived
    WT = big_p.tile([128, NG], F32, tag="WT")
    HT = big_p.tile([128, NG], F32, tag="HT")
    AT = big_p.tile([128, NG], F32, tag="AT")
    vec.tensor_tensor(WT[:, :], TGTC[:, :, 2], TGTC[:, :, 0], OP.subtract)
    vec.tensor_tensor(HT[:, :], TGTC[:, :, 3], TGTC[:, :, 1], OP.subtract)
    vec.tensor_tensor(AT[:, :], WT[:, :], HT[:, :], OP.mult)
    # TRI rows feed lq row2 in q-order: (tx1, tx2, ty1, ty2, at)
    TRI = big_p.tile([128, NG, 5], F32, tag="TRI")
    vec.tensor_copy(
        TRI[:, :, 0:4].rearrange("p g (b a) -> p g b a", a=2),
        TGTC[:, :, :].rearrange("p g (a b) -> p g b a", b=2),
    )
    vec.tensor_copy(TRI[:, :, 4], AT[:, :])

    # persistent lhsT / rhs row tiles (double-buffered by hand so const rows
    # are written once per buffer)
    prs, lqs = [], []
    for half in range(2):
        pr = rows_p.tile([3, NQ, NN], F32R, tag=f"pr{half}", name=f"pr{half}")
        lq = rows_p.tile([3, NQ, 128], F32R, tag=f"lq{half}", name=f"lq{half}")
        nc.sync.dma_start(
            pr[:, :, :], ir_d[:, 0 : NQ * NN].rearrange("x (q n) -> x q n", q=NQ)
        )
        nc.sync.dma_start(
            lq[:, :, :],
            ir_d[:, NQ * NN :].rearrange("x (q n) -> x q n", q=NQ),
        )
        prs.append(pr)
        lqs.append(lq)

    # pre-transpose every group's target rows (TRI) once:
    # TROWS[5*(g%16)+c, g//16, :] holds lq row c for group g
    TROWS = big_p.tile([80, 2, 128], F32R, tag="TROWS")
    for h in range(2):
        tpsB = psB_p.tile([80, 128], F32, tag="tprep")
        nc.tensor.transpose(
            tpsB[:, :],
            TRI[:, 16 * h : 16 * h + 16, :].rearrange("p g c -> p (g c)"),
            ident[:, :],
        )
        vec.tensor_copy(TROWS[:, h, :], tpsB[:, :])

    # manual PSUM banks: q4 alternates halves per chunk; (q5,q6) share one bank
    Q4 = psC_p.tile([128, 512], F32, tag="q4", name="q4bank")
    QT56 = psC_p.tile([128, 512], F32, tag="qt56", name="qt56bank")

    # accumulators
    ACCB = big_p.tile([128, 8], F32, tag="ACCB")     # smooth-l1 partials
    ACCC = big_p.tile([128, 2], F32, tag="ACCC")     # cls partials
    nc.vector.memset(ACCB[:, :], 0.0)
    nc.vector.memset(ACCC[:, :], 0.0)
    CNT = big_p.tile([128, NJ, BC], F32, tag="CNT")  # match counts
    GALL = big_p.tile([128, NG, 16], F32, tag="GALL")

    # ---------------- per-group matching ----------------
    for g in range(NG):
        pr, lq = prs[g % 2], lqs[g % 2]

        # prep: pred rows for both samples, target rows (PE transpose ->
        # SBUF stage -> reshape DMA into row layout)
        for s in range(2):
            tpq = psB_p.tile([80, 128], F32, tag="tprep")
            nc.tensor.transpose(
                tpq[0 : NQ * NJ, :],
                DQ[:, 2 * g + s, :, :].rearrange("p q j -> p (q j)"),
                ident[:, :],
            )
            stq = small_p.tile([NQ * NJ, 128], F32R, tag="stq")
            vec.tensor_copy(stq[:, :], tpq[0 : NQ * NJ, :])
            nc.sync.dma_start(
                pr[s : s + 1, :, :].rearrange("one q (j p) -> one (q j) p", p=128),
                stq[:, :],
            )
        nc.sync.dma_start(
            lq[2:3, 0:5, :], TROWS[5 * (g % 16) : 5 * (g % 16) + 5, g // 16, :]
        )

        S = sbig_p.tile([128, NN], F32, tag="S")
        if KSUB == 1:
            continue
        for ci, (c0, cw) in enumerate(CHUNKS):
            qp0 = psA_p.tile([128, 512], F32, tag="qp0", name="qp0")
            qp1 = psA_p.tile([128, 512], F32, tag="qp1", name="qp1")
            h4 = 256 * ((ci + g) % 2)
            qt = [
                qp0[:, 0:256], qp0[:, 256:512],
                qp1[:, 0:256], qp1[:, 256:512],
                Q4[:, h4 : h4 + 256],
                QT56[:, 0:256], QT56[:, 256:512],
            ]
            for q in range(NQ):
                nc.tensor.matmul(
                    qt[q],
                    lq[:, q, :],
                    pr[:, q, c0 : c0 + cw],
                    start=True,
                    stop=True,
                )
            if KSUB == 2:
                continue
            # ACT evacuates one side of each min from PSUM (>=1-PSUM-operand
            # rule); the wt/ht min rides the first DVE stt, relu the second.
            # min(relu(i1), wt, i2) clamped at 0 == baseline's min of relus
            # because wt,ht,wb,hb > 0.
            r1x = work_p.tile([128, 256], F32, tag="r1x")
            r1y = work_p.tile([128, 256], F32, tag="r1y")
            act.activation(r1x[:, 0:cw], qt[0], AF.Relu)
            act.activation(r1y[:, 0:cw], qt[2], AF.Relu)
            rs = work_p.tile([128, 256], F32, tag="rs")
            vec.reciprocal_approx_fast(rs[:, 0:cw], qt[4])

            if KSUB == 3:
                continue
            mx = work_p.tile([128, 256], F32, tag="mx")
            my = work_p.tile([128, 256], F32, tag="my")
            vec.scalar_tensor_tensor(
                mx[:, 0:cw], r1x[:, 0:cw], WT[:, g : g + 1], qt[1],
                OP.min, OP.min,
            )
            vec.scalar_tensor_tensor(
                my[:, 0:cw], r1y[:, 0:cw], HT[:, g : g + 1], qt[3],
                OP.min, OP.min,
            )
            if KSUB == 4:
                continue
            dxr = work_p.tile([128, 256], F32, tag="dxr")
            dyr = work_p.tile([128, 256], F32, tag="dyr")
            vec.scalar_tensor_tensor(
                dxr[:, 0:cw], mx[:, 0:cw], 0.0, qt[5],
                OP.max, OP.min,
            )
            vec.scalar_tensor_tensor(
                dyr[:, 0:cw], my[:, 0:cw], 0.0, qt[6],
                OP.max, OP.min,
            )
            if KSUB == 5:
                continue
            ip = work_p.tile([128, 256], F32, tag="ip")
            vec.tensor_tensor(ip[:, 0:cw], dxr[:, 0:cw], dyr[:, 0:cw], OP.mult)
            vec.tensor_tensor(S[:, c0 : c0 + cw], ip[:, 0:cw], rs[:, 0:cw], OP.mult)

        vmaxt = small_p.tile([128, 1], F32, tag="vmaxt")
        vec.tensor_reduce(vmaxt[:, :], S[:, :], AX.X, OP.max)
        vmax = vmaxt[:, :]

        if STAGE == 2:
            continue

        # argmax (first occurrence of max = jnp.argmax tie-breaking)
        v8 = small_p.tile([128, 8], F32, tag="v8")
        vec.tensor_scalar(v8[:, :], S[:, 0:8], 0.0, vmax, OP.mult, OP.add)
        idx8 = small_p.tile([128, 8], U32, tag="idx8")
        vec.max_index(idx8[:, :], v8[:, :], S[:, :])
        matchf = small_p.tile([128, 1], F32, tag="matchf")
        vec.tensor_copy(matchf[:, :], idx8[:, 0:1])
        if DEBUG:
            nc.sync.dma_start(dbg_d[:, g : g + 1], matchf[:, :])
            nc.sync.dma_start(dbgs_d[g, :, :], S[:, :])

        if STAGE == 3:
            continue

        # broadcast matched over partitions: transpose -> row -> ones matmul
        mbgat = psB_p.tile([128, 144], F32, tag="mb")
        nc.tensor.transpose(mbgat[0:1, 0:128], matchf[:, :], ident[:, :])
        mrow = small_p.tile([1, 128], F32, tag="mrow")
        act.activation(mrow[:, :], mbgat[0:1, 0:128], AF.Copy)
        nc.tensor.matmul(
            mbgat[:, 0:128], ones1[:, :], mrow[:, :], start=True, stop=True
        )
        MB = small_p.tile([128, 128], F32, tag="MB")
        act.activation(MB[:, :], mbgat[:, 0:128], AF.Copy)

        # gather rhs: GRB[p, j, col]; cols 0..7 = (nbx1,nby1,bx2,by2) x s,
        # cols 8..15 = logits c x s
        GRB = small_p.tile([128, NJ, 16], F32, tag="GRB")
        for qi, q in enumerate((1, 3, 0, 2)):
            vec.tensor_copy(
                GRB[:, :, 2 * qi : 2 * qi + 2],
                DQ[:, 2 * g : 2 * g + 2, q, :].rearrange("p s j -> p j s"),
            )
        vec.tensor_copy(
            GRB[:, :, 8:16].rearrange("p j (c s) -> p j c s", s=2),
            X[:, 2 * g : 2 * g + 2, :, 5:9].rearrange("p s j c -> p j c s"),
        )

        # one-hot M per j tile (+ per-sample match counts), gather matmuls
        gat = mbgat[:, 128:144]
        for j in range(NJ):
            M = small_p.tile([128, 128], F32, tag="M")
            for s in range(2):
                vec.tensor_scalar(
                    M[:, 64 * s : 64 * s + 64],
                    MB[:, 64 * s : 64 * s + 64],
                    njcol[:, j : j + 1],
                    None,
                    OP.is_equal,
                    OP.add,
                    accum_out=CNT[:, j, 2 * g + s : 2 * g + s + 1],
                )
            nc.tensor.matmul(
                gat, M[:, :], GRB[:, j, :], start=(j == 0), stop=(j == NJ - 1)
            )
        act.activation(GALL[:, g, :], gat, AF.Copy)

    if STAGE in (2, 3):
        o2 = small_p.tile([3, 1], F32, tag="o2")
        vec.tensor_copy(o2[:, :], TGTC[0:3, 0, 0:1])
        nc.sync.dma_start(out_d[:].rearrange("(x o) -> x o", o=1), o2[:, :])
        return

    # ---------------- losses ----------------
    # box: smooth-l1 on |g - t|; x1/y1 slots hold -x1 so use add there
    junk = big_p.tile([128, BC * NJ], F32, tag="junk")
    D = big_p.tile([128, NG], F32, tag="D")
    DM = big_p.tile([128, NG], F32, tag="DM")
    Q1 = big_p.tile([128, NG], F32, tag="Q1")
    Q2 = big_p.tile([128, NG], F32, tag="Q2")
    col = 0
    for s in range(2):
        P = slice(64 * s, 64 * s + 64)
        for cc, (q2, op_) in enumerate(
            [(0, OP.add), (1, OP.add), (2, OP.subtract), (3, OP.subtract)]
        ):
            vec.tensor_tensor(D[P, :], GALL[P, :, 2 * q2 + s], TGTC[P, :, cc], op_)
            act.activation(D[P, :], D[P, :], AF.Abs)
            vec.tensor_scalar(DM[P, :], D[P, :], 1.0, None, OP.min)
            act.activation(Q1[P, :], DM[P, :], AF.Square, scale=SQRT_HALF)
            act.activation(Q2[P, :], D[P, :], AF.Relu, bias=b_neg1[P, :])
            vec.scalar_tensor_tensor(
                junk[P, 0:NG], Q1[P, :], 0.0, Q2[P, :], OP.add, OP.add,
                accum_out=ACCB[P, col : col + 1],
            )
            col += 1

    # cls: logsumexp(L) - L[y]  (logits ~ N(0,1): no max-subtraction needed)
    Y = big_p.tile([128, NG, C], F32, tag="Y")
    for cc in range(C):
        vec.tensor_scalar(Y[:, :, cc], TCF[:, :], float(cc), None, OP.is_equal)
    E = big_p.tile([128, NG, C], F32, tag="E")
    SE = big_p.tile([128, NG], F32, tag="SE")
    LSE = big_p.tile([128, NG], F32, tag="LSE")
    ZY = big_p.tile([128, NG, C], F32, tag="ZY")
    SZY = big_p.tile([128, NG], F32, tag="SZY")
    for s in range(2):
        P = slice(64 * s, 64 * s + 64)
        L = GALL[P, :, :].rearrange("p g (q two) -> p g q two", two=2)[:, :, 4:8, s]
        act.activation(E[P, :, :], L, AF.Exp)
        vec.tensor_reduce(SE[P, :], E[P, :, :], AX.X, OP.add)
        act.activation(LSE[P, :], SE[P, :], AF.Ln)
        vec.tensor_tensor(ZY[P, :, :], L, Y[P, :, :], OP.mult)
        vec.tensor_reduce(SZY[P, :], ZY[P, :, :], AX.X, OP.add)
        vec.scalar_tensor_tensor(
            junk[P, 0:NG], LSE[P, :], 0.0, SZY[P, :], OP.add, OP.subtract,
            accum_out=ACCC[P, s : s + 1],
        )

    # conf: sum softplus(x) - sum x*pos; softplus = relu(x) + ln(1+exp(-|x|))
    SP = big_p.tile([128, 1], F32, tag="SP")
    SA = big_p.tile([128, BC * NJ], F32, tag="SA")
    SR = big_p.tile([128, BC * NJ], F32, tag="SR")
    x4flat = X[:, :, :, 4].rearrange("p s j -> p (s j)")
    act.activation(SA[:, :], x4flat, AF.Abs)
    act.activation(SA[:, :], SA[:, :], AF.Exp, scale=-1.0)
    act.activation(SA[:, :], SA[:, :], AF.Ln, bias=1.0)
    act.activation(SR[:, :], x4flat, AF.Relu)
    vec.scalar_tensor_tensor(
        junk[:, :], SA[:, :], 0.0, SR[:, :], OP.add, OP.add, accum_out=SP[:, :]
    )
    POS = big_p.tile([128, NJ, BC], F32, tag="POS")
    vec.tensor_scalar(POS[:, :, :], CNT[:, :, :], 1.0, None, OP.is_ge)
    XP = big_p.tile([128, 1], F32, tag="XP")
    vec.scalar_tensor_tensor(
        junk[:, :].rearrange("p (j s) -> p j s", j=NJ),
        POS[:, :, :],
        0.0,
        X[:, :, :, 4].rearrange("p s j -> p j s"),
        OP.add,
        OP.mult,
        accum_out=XP[:, :],
    )

    # combine partials -> [box, cls, conf] via PE partition reduction
    OV = big_p.tile([128, 3], F32, tag="OV")
    vec.tensor_reduce(OV[:, 0:1], ACCB[:, :], AX.X, OP.add)
    vec.tensor_reduce(OV[:, 1:2], ACCC[:, :], AX.X, OP.add)
    vec.tensor_tensor(OV[:, 2:3], SP[:, :], XP[:, :], OP.subtract)
    red_ps = psB_p.tile([3, 1], F32, tag="mb")
    nc.tensor.matmul(red_ps[:, :], OV[:, :], onescol[:, :], start=True, stop=True)
    outs = small_p.tile([3, 1], F32, tag="outs")
    vec.tensor_copy(outs[:, :], red_ps[:, :])
    nc.sync.dma_start(out_d[:].rearrange("(x o) -> x o", o=1), outs[:, :])


_NC = None


def _get_nc():
    global _NC
    if _NC is None:
        _NC = build_kernel()
    return _NC


def _initrows():
    ir = np.zeros((3, NQ * NN + NQ * 128), dtype=np.float32)
    pr = ir[:, : NQ * NN].reshape(3, NQ, NN)
    lq = ir[:, NQ * NN :].reshape(3, NQ, 128)
    for q in range(NQ):
        pr[2, q, :] = QCONST[q]
        lq[0, q, 0:64] = 1.0
        lq[1, q, 64:128] = 1.0
    return ir


def kernel(predictions, target_boxes, target_classes):
    nc = _get_nc()
    ir = _initrows()
    in_maps = []
    for c in range(NCORES):
        sl = slice(c * BC, (c + 1) * BC)
        in_maps.append(
            {
                "predictions": np.ascontiguousarray(predictions[sl]),
                "target_boxes": np.ascontiguousarray(target_boxes[sl]),
                "target_classes": np.ascontiguousarray(target_classes[sl]),
                "initrows": ir,
            }
        )
    res = run_bass_kernel_spmd(nc, in_maps, list(range(NCORES))).results
    box = np.float64(0.0)
    cls_ = np.float64(0.0)
    conf = np.float64(0.0)
    for c in range(NCORES):
        o = np.asarray(res[c]["out"], dtype=np.float64)
        box += o[0]
        cls_ += o[1]
        conf += o[2]
    total = (5.0 * box + 1.0 * cls_ + conf) / B
    return np.float32(total)



# revision 42
# speedup vs baseline: 2.3401x; 2.3401x over previous
"""DetectionLoss Trainium2 kernel (8-core data parallel), v2.

Per core: 64 samples x 1176 preds x 64 targets.
Matching layout: 32 groups of 2 samples; partitions = interleaved (t, s)
(p = 2*t + s); free dim = preds (real 1176 of padded 1280).

Score: argmax_n IoU == argmax_n ln(relu(dx)*relu(dy)) - ln(ab/256 + at/256)
  dx = min(bx2,tx2) - max(bx1,tx1)  (corner form; target coords are
  per-partition scalars, pred corners are fp16 broadcast matrices built
  once per group by a doubling-cascade DMA).
ACT computes both Ln's (bias folds at/256); S stays f32 so max_index
first-occurrence reproduces jnp.argmax (incl. bf16-tie cases).

Matched box/cls logits: SWDGE gather of raw pred rows from DRAM by the
argmax index; positive conf mask: SWDGE scatter of 1.0 into a DRAM
scratch (duplicate writes == reference scatter semantics), reloaded in
the X layout for the BCE sum.
"""

import os
import numpy as np
from contextlib import ExitStack

import concourse.bass as bass
import concourse.mybir as mybir
from concourse import bacc, tile
from concourse.bass_utils import run_bass_kernel_spmd
from concourse.masks import make_identity

F32 = mybir.dt.float32
F16 = mybir.dt.float16
I32 = mybir.dt.int32
U32 = mybir.dt.uint32
OP = mybir.AluOpType
AF = mybir.ActivationFunctionType
AX = mybir.AxisListType

B, N, T, C = 512, 1176, 64, 4
NCORES = 8
BC = B // NCORES          # samples per core = 64
NG = BC // 2              # groups of 2 samples = 32
NJ = 10                   # pred tiles of 128 (last partial: 24 rows)
NN = NJ * 128             # 1280 (padded row length)
NTAIL = N - 9 * 128       # 24
IMG_W, IMG_H = 1472.0, 832.0
LN16 = float(np.log(np.float64(16.0)))
LNQ = float(np.log(np.float64(16.0)) - np.log(np.float64(64.0)))  # h/128 exp bias
SQRT_HALF = float(np.sqrt(np.float64(0.5)))

DEBUG = bool(int(os.environ.get("KDEBUG", "0")))
STAGE = int(os.environ.get("KSTAGE", "9"))


def build_kernel():
    nc = bacc.Bacc(
        "TRN2",
        target_bir_lowering=False,
        debug=False,
        enable_asserts=False,
        num_devices=NCORES,
    )
    pred_d = nc.dram_tensor("predictions", [BC, N, 9], F32, kind="ExternalInput").ap()
    tb_d = nc.dram_tensor("target_boxes", [BC, T, 4], F32, kind="ExternalInput").ap()
    tc_d = nc.dram_tensor("target_classes", [BC, T], I32, kind="ExternalInput").ap()
    cst_d = nc.dram_tensor("consts", [128, 2 * NG + 128], F32, kind="ExternalInput").ap()
    rows_d = nc.dram_tensor("rowstage", [BC, 5 * NN], F16, kind="Internal").ap()
    out_d = nc.dram_tensor("out", [3], F32, kind="ExternalOutput").ap()
    dbg = {}
    if DEBUG:
        dbg["S"] = nc.dram_tensor("dbgS", [NG, 128, N], F32, kind="ExternalOutput").ap()
        dbg["mf"] = nc.dram_tensor("dbgMF", [128, NG], F32, kind="ExternalOutput").ap()
        dbg["gr"] = nc.dram_tensor("dbgGR", [128, NG, 9], F32, kind="ExternalOutput").ap()
        dbg["bca"] = nc.dram_tensor("dbgBCA", [128, 5, NN], F32, kind="ExternalOutput").ap()
        dbg["ip"] = nc.dram_tensor("dbgIP", [128, N], F32, kind="ExternalOutput").ap()
        dbg["crn"] = nc.dram_tensor("dbgCRN", [128, 2, 5, NJ], F32, kind="ExternalOutput").ap()
        dbg["stq"] = nc.dram_tensor("dbgSTQ", [2, 50, 128], F32, kind="ExternalOutput").ap()

    with tile.TileContext(nc) as tcx:
        with ExitStack() as ctx:
            emit(ctx, tcx, pred_d, tb_d, tc_d, cst_d, rows_d, out_d, dbg)
    nc.compile()
    return nc


def emit(ctx, tcx, pred_d, tb_d, tc_d, cst_d, rows_d, out_d, dbg):
    nc = tcx.nc
    tp = lambda name, bufs, **kw: ctx.enter_context(
        tcx.tile_pool(name=name, bufs=bufs, **kw)
    )
    const_p = tp("const", 1)
    big_p = tp("big", 1)
    bc_p = tp("bcast", 4)
    work_p = tp("work", 2)
    sp_p = tp("spool", 4)
    dt_p = tp("dtmp", 3)
    m_p = tp("mid", 2)
    sml_p = tp("small", 7)
    psB_p = tp("psB", 2, space="PSUM")

    vec = nc.vector
    act = nc.scalar
    pool = nc.gpsimd

    # ---------------- constants ----------------
    ident16 = const_p.tile([128, 128], F16, tag="ident16", name="ident16")
    make_identity(nc, ident16[:, :])
    onescol = const_p.tile([128, 1], F32, tag="onescol", name="onescol")
    vec.memset(onescol[:, :], 1.0)
    cbias = const_p.tile([128, 4], F32, tag="cbias", name="cbias")
    vec.memset(cbias[:, 0:1], LN16)
    vec.memset(cbias[:, 1:2], -IMG_W / 2)
    vec.memset(cbias[:, 2:3], -IMG_H / 2)
    vec.memset(cbias[:, 3:4], LNQ)
    b_ln16, b_wneg, b_hneg, b_lnq = (cbias[:, i : i + 1] for i in range(4))
    cbias2 = const_p.tile([128, 2], F32, tag="cbias2", name="cbias2")
    vec.memset(cbias2[:, 0:1], -1.0)
    vec.memset(cbias2[:, 1:2], 1e-35)
    b_neg1, b_eps = cbias2[:, 0:1], cbias2[:, 1:2]
    # consts: BASEG[p,g] = (2g + p//64)*N; IDXT[p,g] = 2g + p//64;
    # TRI[p,c] = 1 if c < p and same 64-half (first-occurrence dedup mask)
    CST = const_p.tile([128, 2 * NG + 128], F32, tag="CST", name="CST")
    nc.sync.dma_start(CST[:, :], cst_d[:, :])
    BASEG = CST[:, 0:NG]
    TRIc = CST[:, 2 * NG : 2 * NG + 128]
    IDXT = const_p.tile([128, NG], I32, tag="IDXT", name="IDXT")
    vec.tensor_copy(IDXT[:, :], CST[:, NG : 2 * NG])
    ident32 = const_p.tile([128, 128], F32, tag="ident32", name="ident32")
    make_identity(nc, ident32[:, :])
    ones1 = const_p.tile([1, 128], F32, tag="ones1", name="ones1")
    vec.memset(ones1[:, :], 1.0)

    # ---------------- input loads ----------------
    # X5[p, s, j, k] = pred[s, 128j+p, k] for k<5; pads zero, conf pad -100
    X5 = big_p.tile([128, BC, NJ, 5], F32, tag="X5", name="X5")
    vec.memset(X5[:, :, 9, :], 0.0)
    vec.memset(X5[:, :, 9, 4], -100.0)
    for i in range(2):
        s0 = i * (BC // 2)
        for j in range(NJ):
            pw = 128 if j < 9 else NTAIL
            eng = [nc.sync, nc.scalar][(i * NJ + j) % 2]
            eng.dma_start(
                X5[0:pw, s0 : s0 + BC // 2, j, :],
                pred_d[s0 : s0 + BC // 2, j * 128 : j * 128 + pw, 0:5].rearrange(
                    "s p k -> p s k"
                ),
            )
    # targets, interleaved partitions p = 2t + s
    TGTC = big_p.tile([128, NG, 4], F32, tag="TGTC", name="TGTC")
    nc.sync.dma_start(TGTC[:, :, :], tb_d.rearrange("(g s) t c -> (s t) g c", s=2))
    TCI = big_p.tile([128, NG], I32, tag="TCI", name="TCI")
    nc.sync.dma_start(TCI[:, :], tc_d.rearrange("(g s) t -> (s t) g", s=2))
    TCF = big_p.tile([128, NG], F32, tag="TCF", name="TCF")
    vec.tensor_copy(TCF[:, :], TCI[:, :])

    # ---------------- decode to fp16 corners ----------------
    # CRN[p, s, q, j]: q = (bx1, bx2, by1, by2, ab/256)
    CRN = big_p.tile([128, BC, 5, NJ], F16, tag="CRN", name="CRN")

    def decode_chunk(c):
        SC2 = BC
        sl = slice(0, BC)
        CX = dt_p.tile([128, SC2, NJ], F32, tag="dt", name="CX")
        WHX = dt_p.tile([128, SC2, NJ], F32, tag="dt", name="WHX")
        act.activation(WHX[:, :, :], X5[:, sl, :, 2], AF.Exp, bias=b_ln16, scale=1.0)
        act.activation(
            CX[:, :, :], X5[:, sl, :, 0], AF.Identity, bias=b_wneg, scale=IMG_W
        )
        vec.tensor_tensor(CRN[:, sl, 0, :], CX[:, :, :], WHX[:, :, :], OP.subtract)
        vec.tensor_tensor(CRN[:, sl, 1, :], CX[:, :, :], WHX[:, :, :], OP.add)
        WHQ = dt_p.tile([128, SC2, NJ], F32, tag="dt", name="WHQ")
        act.activation(WHQ[:, :, :], X5[:, sl, :, 3], AF.Exp, bias=b_lnq, scale=1.0)
        # ab/256 = (w/2)*(h/128)
        vec.tensor_tensor(CRN[:, sl, 4, :], WHX[:, :, :], WHQ[:, :, :], OP.mult)
        CY = dt_p.tile([128, SC2, NJ], F32, tag="dt", name="CY")
        WHY = dt_p.tile([128, SC2, NJ], F32, tag="dt", name="WHY")
        act.activation(WHY[:, :, :], X5[:, sl, :, 3], AF.Exp, bias=b_ln16, scale=1.0)
        act.activation(
            CY[:, :, :], X5[:, sl, :, 1], AF.Identity, bias=b_hneg, scale=IMG_H
        )
        vec.tensor_tensor(CRN[:, sl, 2, :], CY[:, :, :], WHY[:, :, :], OP.subtract)
        vec.tensor_tensor(CRN[:, sl, 3, :], CY[:, :, :], WHY[:, :, :], OP.add)

    if DEBUG:
        crnf = big_p.tile([128, 2, 5, NJ], F32, tag="crnf", name="crnf")
        vec.tensor_copy(crnf[:, :, :, :], CRN[:, 0:2, :, :])
        nc.sync.dma_start(dbg["crn"][:, :, :, :], crnf[:, :, :, :])

    # target-derived per-partition scalars
    WT = big_p.tile([128, NG], F32, tag="WT", name="WT")
    HT = big_p.tile([128, NG], F32, tag="HT", name="HT")
    AT = big_p.tile([128, NG], F32, tag="AT", name="AT")
    vec.tensor_tensor(WT[:, :], TGTC[:, :, 2], TGTC[:, :, 0], OP.subtract)
    vec.tensor_tensor(HT[:, :], TGTC[:, :, 3], TGTC[:, :, 1], OP.subtract)
    vec.tensor_tensor(AT[:, :], WT[:, :], HT[:, :], OP.mult)
    AT256 = big_p.tile([128, NG], F32, tag="AT256", name="AT256")
    vec.tensor_scalar(AT256[:, :], AT[:, :], 1.0 / 256.0, None, OP.mult)

    if STAGE == 0:
        o0 = sml_p.tile([3, 1], F32, tag="o0", name="o0")
        vec.tensor_copy(o0[:, :], CRN[0:3, 0, 0, 0:1])
        nc.sync.dma_start(out_d[:].rearrange("(x o) -> x o", o=1), o0[:, :])
        return

    # stage per-sample corner rows to DRAM: rows_d[s, q*NN + (j*128+p)]
    def stage_sample(s):
        tpq = psB_p.tile([50, 128], F16, tag="tpq", name="tpq")
        nc.tensor.transpose(
            tpq[:, :],
            CRN[:, s, :, :].rearrange("p q j -> p (q j)"),
            ident16[:, :],
        )
        stq = m_p.tile([50, 128], F16, tag="stq", name="stq", bufs=6)
        act.activation(stq[:, :], tpq[:, :], AF.Copy)
        eng = [nc.sync, nc.scalar][s % 2]
        eng.dma_start(
            rows_d[s : s + 1, :].rearrange("one (q j p) -> one (q j) p", p=128, j=NJ),
            stq[:, :],
        )

    decode_chunk(0)
    for s in range(BC):
        stage_sample(s)

    # accumulators
    ACCB = big_p.tile([128, 1], F32, tag="ACCB", name="ACCB")
    ACCC = big_p.tile([128, 1], F32, tag="ACCC", name="ACCC")
    FO = big_p.tile([128, NG], F32, tag="FO", name="FO")
    IDX32 = big_p.tile([128, NG], I32, tag="IDX32", name="IDX32")
    GR = big_p.tile([128, NG, 9], F32, tag="GR", name="GR")
    junk = big_p.tile([128, 640], F32, tag="junk", name="junk")
    junkp = big_p.tile([128, 128], F32, tag="junkp", name="junkp")
    SPB = big_p.tile([128, 640], F32, tag="SPB", name="SPB")
    SRB = big_p.tile([128, 640], F32, tag="SRB", name="SRB")
    SP = big_p.tile([128, 1], F32, tag="SP", name="SP")
    Y = big_p.tile([128, NG, C], F32, tag="Y", name="Y")
    predflat = pred_d.rearrange("s n k -> (s n) k")

    # ---------------- per-group matching (software-pipelined) ----------------
    # phases per group g:
    #   A(g): broadcast gather (pool queue; prefetched 2 groups ahead)
    #   B1(g): lnsab + fp16 min/max chain -> ip -> lnip -> S (DVE/ACT/pool)
    #   B2(g): vmax/v8/max_index (DVE; lagged 1 group so DVE stays fed)
    #   C(g): mf/dedup/idx/gather tail (lagged 2 groups; off critical path)
    st_ = {}

    def phase_a(g):
        BCA = bc_p.tile([128, 5, NN], F16, tag="BCA", name="BCA")
        pool.indirect_dma_start(
            out=BCA[:, :, :].rearrange("p q n -> p (q n)"),
            out_offset=None,
            in_=rows_d[:, :],
            in_offset=bass.IndirectOffsetOnAxis(ap=IDXT[:, g : g + 1], axis=0),
        )
        st_[("bca", g)] = BCA

    def phase_b1(g):
        BCA = st_[("bca", g)]
        LNSAB = work_p.tile([128, N], F32, tag="LNSAB", name="LNSAB", bufs=3)
        act.activation(
            LNSAB[:, :], BCA[:, 4, 0:N], AF.Ln, bias=AT256[:, g : g + 1], scale=1.0
        )
        tx1 = TGTC[:, g, 0:1]
        ty1 = TGTC[:, g, 1:2]
        tx2 = TGTC[:, g, 2:3]
        ty2 = TGTC[:, g, 3:4]
        PQ2 = work_p.tile([128, 2, N], F16, tag="PQ2", name="PQ2")
        PQ1 = work_p.tile([128, 2, N], F16, tag="PQ1", name="PQ1")
        vec.tensor_scalar(PQ2[:, 0, :], BCA[:, 1, 0:N], tx2, None, OP.min)
        vec.tensor_scalar(PQ2[:, 1, :], BCA[:, 3, 0:N], ty2, None, OP.min)
        vec.tensor_scalar(PQ1[:, 0, :], BCA[:, 0, 0:N], tx1, None, OP.max)
        vec.tensor_scalar(PQ1[:, 1, :], BCA[:, 2, 0:N], ty1, None, OP.max)
        DXY = work_p.tile([128, 2, N], F16, tag="DXY", name="DXY")
        vec.tensor_tensor(
            DXY[:, :, :].rearrange("p a n -> p (a n)"),
            PQ2[:, :, :].rearrange("p a n -> p (a n)"),
            PQ1[:, :, :].rearrange("p a n -> p (a n)"),
            OP.subtract,
        )
        RXY = work_p.tile([128, 2, N], F16, tag="RXY", name="RXY")
        vec.tensor_scalar(
            RXY[:, :, :].rearrange("p a n -> p (a n)"),
            DXY[:, :, :].rearrange("p a n -> p (a n)"),
            0.0,
            None,
            OP.max,
        )
        IP = work_p.tile([128, N], F16, tag="IP", name="IP")
        vec.tensor_tensor(IP[:, :], RXY[:, 0, :], RXY[:, 1, :], OP.mult)
        LNIP = work_p.tile([128, N], F32, tag="LNIP", name="LNIP", bufs=3)
        act.activation(LNIP[:, :], IP[:, :], AF.Ln, bias=b_eps, scale=1.0)
        S = sp_p.tile([128, N], F32, tag="S", name="S")
        pool.tensor_tensor(S[:, :], LNIP[:, :], LNSAB[:, :], OP.subtract)
        st_[("s", g)] = S
        if DEBUG:
            nc.sync.dma_start(dbg["S"][g, :, :], S[:, :])

    def phase_b2(g):
        S = st_[("s", g)]
        vmax = sml_p.tile([128, 1], F32, tag="vmax", name="vmax")
        vec.tensor_reduce(vmax[:, :], S[:, :], AX.X, OP.max)
        v8 = sml_p.tile([128, 8], F32, tag="v8", name="v8")
        vec.tensor_scalar(v8[:, :], S[:, 0:8], 0.0, vmax[:, :], OP.mult, OP.add)
        idx8 = sml_p.tile([128, 8], U32, tag="idx8", name="idx8")
        vec.max_index(idx8[:, :], v8[:, :], S[:, :])
        st_[("vmax", g)] = vmax
        st_[("idx8", g)] = idx8

    def phase_c(g):
        vmax = st_.pop(("vmax", g))
        idx8 = st_.pop(("idx8", g))
        st_.pop(("s", g))
        st_.pop(("bca", g))
        # zero-overlap rows (vmax < -35 in log space) must match pred 0 to
        # reproduce jnp.argmax over an all-zero IoU row.
        mf = sml_p.tile([128, 1], F32, tag="mf", name="mf")
        pool.tensor_copy(mf[:, :], idx8[:, 0:1])
        msk = sml_p.tile([128, 1], F32, tag="msk", name="msk")
        pool.tensor_scalar(msk[:, :], vmax[:, :], -35.0, None, OP.is_gt)
        pool.tensor_tensor(mf[:, :], mf[:, :], msk[:, :], OP.mult)
        idxg = sml_p.tile([128, 1], F32, tag="idxg", name="idxg")
        pool.tensor_scalar(idxg[:, :], mf[:, :], BASEG[:, g : g + 1], None, OP.add)
        pool.tensor_copy(IDX32[:, g : g + 1], idxg[:, :])
        # gather the matched raw pred row (box params + conf + cls logits)
        pool.indirect_dma_start(
            out=GR[:, g, :],
            out_offset=None,
            in_=predflat[:, :],
            in_offset=bass.IndirectOffsetOnAxis(ap=IDX32[:, g : g + 1], axis=0),
        )
        # first-occurrence dedup of matched preds within each sample half:
        # FO[p] = 1 iff no earlier partition in the same half matched the
        # same pred (reproduces the reference scatter's positive-mask dedup)
        tpm = psB_p.tile([1, 128], F32, tag="tpm", name="tpm")
        nc.tensor.transpose(tpm[:, :], mf[:, :], ident32[:, :])
        mrow = m_p.tile([1, 128], F32, tag="mrow", name="mrow")
        act.activation(mrow[:, :], tpm[:, :], AF.Copy)
        mbp = psB_p.tile([128, 128], F32, tag="mbp", name="mbp")
        nc.tensor.matmul(mbp[:, :], ones1[:, :], mrow[:, :], start=True, stop=True)
        mbs = m_p.tile([128, 128], F32, tag="mbs", name="mbs")
        act.activation(mbs[:, :], mbp[:, :], AF.Copy)
        cntc = sml_p.tile([128, 1], F32, tag="cntc", name="cntc")
        vec.scalar_tensor_tensor(
            junkp[:, :], mbs[:, :], mf[:, :], TRIc[:, :],
            OP.is_equal, OP.mult, accum_out=cntc[:, :],
        )
        pool.tensor_scalar(FO[:, g : g + 1], cntc[:, :], 0.5, None, OP.is_lt)

    phase_a(0)
    phase_a(1)
    phase_a(2)
    for g in range(NG):
        if g + 3 < NG:
            phase_a(g + 3)
        if STAGE >= 2:
            phase_b1(g)
        if STAGE >= 3 and g >= 3:
            phase_b2(g - 3)
        if STAGE >= 4 and g >= 4:
            phase_c(g - 4)
        if g == 8 and STAGE >= 4:
            # conf softplus sum: softplus(x) = relu(x) + ln(1 + exp(-|x|));
            # independent of matching - hoisted into loop slack
            x4h = X5[:, :, :, 4].rearrange("p s j -> p (s j)")
            act.activation(SPB[:, :], x4h, AF.Abs)
            act.activation(SPB[:, :], SPB[:, :], AF.Exp, scale=-1.0)
            act.activation(SPB[:, :], SPB[:, :], AF.Ln, bias=1.0)
            act.activation(SRB[:, :], x4h, AF.Relu)
            vec.scalar_tensor_tensor(
                junk[:, :], SPB[:, :], 0.0, SRB[:, :], OP.add, OP.add,
                accum_out=SP[:, :],
            )
        if g == 12 and STAGE >= 4:
            for cc in range(C):
                vec.tensor_scalar(Y[:, :, cc], TCF[:, :], float(cc), None, OP.is_equal)
    if STAGE >= 3:
        phase_b2(NG - 3)
        phase_b2(NG - 2)
        phase_b2(NG - 1)
    if STAGE >= 4:
        for g in range(NG - 4, NG):
            phase_c(g)

    if STAGE in (1, 2, 3):
        if DEBUG and STAGE == 3:
            mfd = big_p.tile([128, NG], F32, tag="mfd", name="mfd")
            vec.tensor_copy(mfd[:, :], IDX32[:, :])
            nc.sync.dma_start(dbg["mf"][:, :], mfd[:, :])
        o1 = sml_p.tile([3, 1], F32, tag="o1", name="o1")
        vec.tensor_copy(o1[:, :], TGTC[0:3, 0, 0:1])
        nc.sync.dma_start(out_d[:].rearrange("(x o) -> x o", o=1), o1[:, :])
        return

    # ---------------- losses ----------------
    if DEBUG:
        mfdbg = big_p.tile([128, NG], F32, tag="mfdbg", name="mfdbg")
        vec.tensor_copy(mfdbg[:, :], IDX32[:, :])
        nc.sync.dma_start(dbg["mf"][:, :], mfdbg[:, :])
        nc.sync.dma_start(dbg["gr"][:, :, :], GR[:, :, :])

    if STAGE == 4:
        o4 = sml_p.tile([3, 1], F32, tag="o4", name="o4")
        vec.tensor_copy(o4[:, :], GR[0:3, 0, 0:1])
        nc.sync.dma_start(out_d[:].rearrange("(x o) -> x o", o=1), o4[:, :])
        return

    # conf positive sum: deduped matched conf logits
    XP = big_p.tile([128, 1], F32, tag="XP", name="XP")
    vec.scalar_tensor_tensor(
        junk[:, 0:NG], FO[:, :], 0.0, GR[:, :, 4], OP.add, OP.mult,
        accum_out=XP[:, :],
    )

    # box loss (smooth l1) on gathered + re-decoded corners
    CXM = big_p.tile([128, NG], F32, tag="CXM", name="CXM")
    CYM = big_p.tile([128, NG], F32, tag="CYM", name="CYM")
    WXM = big_p.tile([128, NG], F32, tag="WXM", name="WXM")
    WYM = big_p.tile([128, NG], F32, tag="WYM", name="WYM")
    vec.tensor_scalar(CXM[:, :], GR[:, :, 0], IMG_W, -IMG_W / 2, OP.mult, OP.add)
    vec.tensor_scalar(CYM[:, :], GR[:, :, 1], IMG_H, -IMG_H / 2, OP.mult, OP.add)
    act.activation(WXM[:, :], GR[:, :, 2], AF.Exp, bias=b_ln16, scale=1.0)
    act.activation(WYM[:, :], GR[:, :, 3], AF.Exp, bias=b_ln16, scale=1.0)
    B4 = big_p.tile([128, NG, 4], F32, tag="B4", name="B4")
    vec.tensor_tensor(B4[:, :, 0], CXM[:, :], WXM[:, :], OP.subtract)
    vec.tensor_tensor(B4[:, :, 1], CYM[:, :], WYM[:, :], OP.subtract)
    vec.tensor_tensor(B4[:, :, 2], CXM[:, :], WXM[:, :], OP.add)
    vec.tensor_tensor(B4[:, :, 3], CYM[:, :], WYM[:, :], OP.add)
    # reference order is (x1,y1,x2,y2); B4 is (x1,y1,x2,y2) via cols 0..3
    D = big_p.tile([128, NG, 4], F32, tag="D", name="D")
    vec.tensor_tensor(
        D[:, :, :].rearrange("p g c -> p (g c)"),
        B4[:, :, :].rearrange("p g c -> p (g c)"),
        TGTC[:, :, :].rearrange("p g c -> p (g c)"),
        OP.subtract,
    )
    AD = big_p.tile([128, NG, 4], F32, tag="AD", name="AD")
    act.activation(
        AD[:, :, :].rearrange("p g c -> p (g c)"),
        D[:, :, :].rearrange("p g c -> p (g c)"),
        AF.Abs,
    )
    DM = big_p.tile([128, NG, 4], F32, tag="DM", name="DM")
    adf = AD[:, :, :].rearrange("p g c -> p (g c)")
    vec.tensor_scalar(DM[:, :, :].rearrange("p g c -> p (g c)"), adf, 1.0, None, OP.min)
    Q1 = big_p.tile([128, NG, 4], F32, tag="Q1", name="Q1")
    Q2 = big_p.tile([128, NG, 4], F32, tag="Q2", name="Q2")
    act.activation(
        Q1[:, :, :].rearrange("p g c -> p (g c)"),
        DM[:, :, :].rearrange("p g c -> p (g c)"),
        AF.Square,
        scale=SQRT_HALF,
    )
    act.activation(
        Q2[:, :, :].rearrange("p g c -> p (g c)"), adf, AF.Relu, bias=b_neg1, scale=1.0
    )
    vec.scalar_tensor_tensor(
        junk[:, 0 : NG * 4],
        Q1[:, :, :].rearrange("p g c -> p (g c)"),
        0.0,
        Q2[:, :, :].rearrange("p g c -> p (g c)"),
        OP.add,
        OP.add,
        accum_out=ACCB[:, :],
    )

    # cls loss: logsumexp(L) - L[y]
    E = big_p.tile([128, NG, C], F32, tag="E", name="E")
    SE = big_p.tile([128, NG], F32, tag="SE", name="SE")
    LSE = big_p.tile([128, NG], F32, tag="LSE", name="LSE")
    ZY = big_p.tile([128, NG, C], F32, tag="ZY", name="ZY")
    SZY = big_p.tile([128, NG], F32, tag="SZY", name="SZY")
    act.activation(E[:, :, :], GR[:, :, 5:9], AF.Exp)
    vec.tensor_reduce(SE[:, :], E[:, :, :], AX.X, OP.add)
    act.activation(LSE[:, :], SE[:, :], AF.Ln)
    vec.tensor_tensor(ZY[:, :, :], GR[:, :, 5:9], Y[:, :, :], OP.mult)
    vec.tensor_reduce(SZY[:, :], ZY[:, :, :], AX.X, OP.add)
    vec.scalar_tensor_tensor(
        junk[:, 0:NG],
        LSE[:, :],
        0.0,
        SZY[:, :],
        OP.add,
        OP.subtract,
        accum_out=ACCC[:, :],
    )

    # combine -> [box, cls, conf] via PE partition reduction
    OV = big_p.tile([128, 3], F32, tag="OV", name="OV")
    vec.tensor_copy(OV[:, 0:1], ACCB[:, :])
    vec.tensor_copy(OV[:, 1:2], ACCC[:, :])
    vec.tensor_tensor(OV[:, 2:3], SP[:, :], XP[:, :], OP.subtract)
    red_ps = psB_p.tile([3, 1], F32, tag="red", name="red")
    nc.tensor.matmul(red_ps[:, :], OV[:, :], onescol[:, :], start=True, stop=True)
    outs = sml_p.tile([3, 1], F32, tag="outs", name="outs")
    vec.tensor_copy(outs[:, :], red_ps[:, :])
    nc.sync.dma_start(out_d[:].rearrange("(x o) -> x o", o=1), outs[:, :])


_NC = None


def _get_nc():
    global _NC
    if _NC is None:
        _NC = build_kernel()
    return _NC


def _consts():
    p = np.arange(128, dtype=np.float32)[:, None]
    g = np.arange(NG, dtype=np.float32)[None, :]
    baseg = (2.0 * g + np.floor(p / 64.0)) * np.float32(N)
    idxt = 2.0 * g + np.floor(p / 64.0)
    pi = np.arange(128)[:, None]
    ci = np.arange(128)[None, :]
    tri = ((ci < pi) & ((ci // 64) == (pi // 64))).astype(np.float32)
    return np.concatenate([baseg, idxt, tri], axis=1).astype(np.float32)


def kernel(predictions, target_boxes, target_classes):
    nc = _get_nc()
    cst = _consts()
    in_maps = []
    for c in range(NCORES):
        sl = slice(c * BC, (c + 1) * BC)
        in_maps.append(
            {
                "predictions": np.ascontiguousarray(predictions[sl]),
                "target_boxes": np.ascontiguousarray(target_boxes[sl]),
                "target_classes": np.ascontiguousarray(target_classes[sl]),
                "consts": cst,
            }
        )
    res = run_bass_kernel_spmd(nc, in_maps, list(range(NCORES))).results
    box = np.float64(0.0)
    cls_ = np.float64(0.0)
    conf = np.float64(0.0)
    for c in range(NCORES):
        o = np.asarray(res[c]["out"], dtype=np.float64)
        box += o[0]
        cls_ += o[1]
        conf += o[2]
    total = (5.0 * box + 1.0 * cls_ + conf) / B
    return np.float32(total)
